# revision 2
# baseline (speedup 1.0000x reference)
"""CondConv2d (MoE-routed 3x3 conv) Trainium2 Bass kernel.

Problem (hardcoded shapes):
  x:       (16, 128, 128, 128) f32   B, C_in, H, W
  experts: (4, 128, 128, 3, 3) f32   K, C_out, C_in, kh, kw
  bias:    (4, 128) f32              K, C_out
  w1:      (32, 128) f32             HID, C_in
  b1:      (32,) f32
  w2:      (4, 32) f32               K, HID
  b2:      (4,) f32
  out:     (16, 128, 128, 128) f32   B, C_out, H, W  (stride 1, pad 1)

Sharding: data-parallel over batch, 2 samples per core x 8 cores; the tiny
expert/router params are replicated (pre-transposed on the host into the
matmul-friendly layouts -- pure layout prep, all math stays on device).

Per-core schedule (PE-bound at ~216ns per 512-col fp16 matmul):
  0-24us   x0 streams in (DMA-bound).  A dense stream of zero matmuls keeps
           the PE HAM activity window busy so the conv starts at 2.4GHz
           instead of 1.2GHz.  DVE chases slabs with channel sums, ACT casts
           fp32->fp16 into the padded image.  x1 slab DMAs are issued up
           front too: the sync ring drains them right after x0's.
  ~26us    router-0 (tiny PE matmuls) -> weff-0 on DVE, split into two tap
           halves so the first conv matmuls start ~2us earlier.
  26-88us  conv-0: 32 chunks x 9 accumulating matmuls (PSUM), evacuation
           alternates ACT/DVE, output DMA per 2 chunks on the idle GpSimd
           ring (a dynamic-DMA trigger costs ~0.8us of issuing-engine time,
           which previously saturated ACT).  x1's reduce/cast ops are
           interleaved into the conv program order so they never block
           conv evacuations on the FIFO engines; router-1 runs mid-stream
           after chunk 16 (x1 has landed by then), so the PE never waits.
  88-152us conv-1, same structure.
"""

import numpy as np

import concourse.bass as bass
import concourse.mybir as mybir
import concourse.tile as tile
from concourse import bass_utils


def _legalize_waits(nc, keep=1):
    """This container's walrus rejects >1 sync wait per instruction
    (setupSyncWait: "Too many sync wait commands").  Hoist extra waits into
    standalone EventSemaphore wait-nops on the same engine, which is what
    raw-bass wait_ge() emits; ">=" waits commute so order doesn't matter."""
    counter = [0]

    def fix_block(block):
        out, changed = [], False
        for inst in block.instructions:
            si = inst.sync_info
            waits = list(si.on_wait) if si is not None else []
            if len(waits) > keep:
                for w in waits[:-keep]:
                    nm = f"{inst.name}-w{counter[0]}"
                    counter[0] += 1
                    nop = mybir.InstEventSemaphore(name=nm, ins=[], outs=[])
                    nop.engine = inst.engine
                    nop.sync_info = mybir.SyncInfo(on_wait=[w], on_update=[])
                    nc.inst_map[nm] = nop
                    out.append(nop)
                inst.sync_info = mybir.SyncInfo(
                    on_wait=waits[-keep:], on_update=list(si.on_update)
                )
                changed = True
            out.append(inst)
        if changed:
            block.instructions = out
        for sub in getattr(block, "blocks", []) or []:
            fix_block(sub)

    for fn in nc.m.functions:
        for b in fn.blocks:
            fix_block(b)


F32 = mybir.dt.float32
F16 = mybir.dt.float16
AF = mybir.ActivationFunctionType
ALU = mybir.AluOpType

B, CIN, COUT, K, KS, H, W, HID = 16, 128, 128, 4, 3, 128, 128, 32
N_CORES = 8
BPC = B // N_CORES          # samples per core
HP, WP = H + 2, W + 2       # zero-padded image
RPC = 4                     # output rows per chunk
NCHUNK = H // RPC           # 32 chunks per sample
FREE = RPC * W              # 512 = matmul moving free size (one PSUM bank)
NSLAB = 8                   # x-load slabs per sample (16 rows each)
SLAB_ROWS = H // NSLAB
JT = KS * KS                # 9 taps
JA = 5                      # taps in the first weff half
WARMUP_MMS = 100            # junk matmuls covering the x0 load window


def build_nc() -> bass.Bass:
    nc = bass.Bass(trn_type="TRN2", target_bir_lowering=False, debug=False)

    x_d = nc.dram_tensor("x", [BPC, CIN, H, W], F32, kind="ExternalInput")
    et_d = nc.dram_tensor("experts_t", [CIN, K, JT, COUT], F32,
                          kind="ExternalInput")
    biast_d = nc.dram_tensor("bias_t", [COUT, K], F32, kind="ExternalInput")
    w1t_d = nc.dram_tensor("w1t", [CIN, HID], F32, kind="ExternalInput")
    b1_d = nc.dram_tensor("b1", [HID], F32, kind="ExternalInput")
    w2t_d = nc.dram_tensor("w2t", [HID, K], F32, kind="ExternalInput")
    b2_d = nc.dram_tensor("b2", [K], F32, kind="ExternalInput")
    y_d = nc.dram_tensor("y", [BPC, COUT, H, W], F32, kind="ExternalOutput")

    with tile.TileContext(nc) as tc:
        with (
            tc.tile_pool(name="singles", bufs=1) as singles,
            tc.tile_pool(name="stage", bufs=4) as stage_pool,
            tc.tile_pool(name="outp", bufs=4) as outp,
            tc.tile_pool(name="pconv", bufs=6, space="PSUM") as pconv,
            tc.tile_pool(name="prt", bufs=2, space="PSUM") as prt,
        ):
            xpads = [None] * BPC
            weffs = [None] * BPC      # (wfA taps 0..4, wfB taps 5..8)
            alphas = [None] * BPC
            rSs = [None] * BPC
            partials_t = [None] * BPC
            beff = singles.tile([COUT, BPC], F32)

            # ---- PE warmup: dense junk matmuls under the x0 load ---------
            warm_w = singles.tile([CIN, COUT], F16)
            warm_x = singles.tile([CIN, FREE], F16)
            nc.vector.memset(warm_w, 0.0)
            nc.vector.memset(warm_x, 0.0)
            for _ in range(WARMUP_MMS):
                wps = pconv.tile([COUT, FREE], F32, tag="ps")
                nc.tensor.matmul(wps, warm_w, warm_x)

            def make_xpad(b):
                xp = singles.tile([CIN, HP, WP], F16, tag=f"xpad{b}",
                                  name=f"xpad{b}")
                xpads[b] = xp
                nc.vector.memset(xp[:, 0, :], 0.0)
                nc.vector.memset(xp[:, HP - 1, :], 0.0)
                nc.vector.memset(xp[:, :, 0], 0.0)
                nc.vector.memset(xp[:, :, WP - 1], 0.0)
                partials_t[b] = singles.tile(
                    [CIN, NSLAB], F32, tag=f"partials{b}", name=f"partials{b}")

            def issue_slab_dmas(b):
                """Queue all of sample b's slab DMAs on the sync ring; the
                ring drains them in order (stage-pool reuse throttles via
                HWDGE semaphore waits)."""
                stages = []
                for s in range(NSLAB):
                    r0 = s * SLAB_ROWS
                    stage = stage_pool.tile([CIN, SLAB_ROWS, W], F32,
                                            tag="stage")
                    nc.sync.dma_start(
                        out=stage, in_=x_d[b, :, r0:r0 + SLAB_ROWS, :])
                    stages.append(stage)
                return stages

            def prep_slab(b, s, stage):
                """DVE channel-sum + ACT fp32->fp16 cast into the padded
                image for one landed slab."""
                r0 = s * SLAB_ROWS
                nc.vector.tensor_reduce(
                    out=partials_t[b][:, s:s + 1], in_=stage,
                    axis=mybir.AxisListType.XY, op=ALU.add)
                nc.scalar.activation(
                    out=xpads[b][:, 1 + r0:1 + r0 + SLAB_ROWS, 1:1 + W],
                    in_=stage, func=AF.Copy)

            def route(b):
                """Router MLP + softmax through broadcast alpha."""
                partials = partials_t[b]
                gT = singles.tile([CIN, 1], F32, tag=f"gT{b}", name=f"gT{b}")
                nc.vector.tensor_reduce(
                    out=gT, in_=partials, axis=mybir.AxisListType.X,
                    op=ALU.add)

                h_ps = prt.tile([HID, 1], F32, tag="rt")
                nc.tensor.matmul(h_ps, w1t, gT)
                h_sb = singles.tile([HID, 1], F32, tag=f"h_sb{b}",
                                    name=f"h_sb{b}")
                nc.scalar.activation(out=h_sb, in_=h_ps, func=AF.Relu,
                                     bias=b1t)

                lg_ps = prt.tile([K, 1], F32, tag="rt")
                nc.tensor.matmul(lg_ps, w2t, h_sb)
                # expl = exp(logits + b2); logits are tiny, no max-sub needed
                expl = singles.tile([K, 1], F32, tag=f"expl{b}",
                                    name=f"expl{b}")
                nc.scalar.activation(out=expl, in_=lg_ps, func=AF.Exp,
                                     bias=b2t)

                # broadcast expl[k] (unnormalized) to all partitions via
                # selector matmuls; the softmax 1/sum normalization is
                # applied later as the PSUM-evacuation scale
                ab_ps = prt.tile([128, K], F32, tag="rt")
                for k in range(K):
                    nc.tensor.matmul(ab_ps[:, k:k + 1], sel[:, k, :], expl)
                alpha = singles.tile([128, K], F32, tag=f"alpha{b}",
                                     name=f"alpha{b}")
                nc.vector.tensor_copy(out=alpha, in_=ab_ps)
                alphas[b] = alpha

                den_ps = prt.tile([128, 1], F32, tag="rt")
                nc.tensor.matmul(den_ps, ones4, expl)
                rS = singles.tile([128, 1], F32, tag=f"rS{b}", name=f"rS{b}")
                nc.vector.reciprocal(out=rS, in_=den_ps)
                rSs[b] = rS

            def weff_block(b):
                """weff_b = sum_k alpha[k] * expertT_k (fp32 accumulate,
                fp16 result), built in two tap halves so conv tap-0 matmuls
                can start before the full chain finishes."""
                alpha = alphas[b]
                halves = []
                for (name, j0, j1) in (("A", 0, JA), ("B", JA, JT)):
                    nj = j1 - j0
                    wf = singles.tile([CIN, nj, COUT], F16,
                                      tag=f"weff{name}{b}",
                                      name=f"weff{name}{b}")
                    wf_flat = wf.rearrange("p j co -> p (j co)")
                    src = et_flat[:, :, j0 * COUT:j1 * COUT]
                    with nc.allow_low_precision(reason="fp16 conv weights"):
                        nc.vector.tensor_scalar_mul(
                            wf_flat, src[:, 0, :], alpha[:, 0:1])
                        for k in range(1, K):
                            nc.vector.scalar_tensor_tensor(
                                out=wf_flat, in0=src[:, k, :],
                                scalar=alpha[:, k:k + 1], in1=wf_flat,
                                op0=ALU.mult, op1=ALU.add)
                    halves.append(wf)
                weffs[b] = halves

                btmp = singles.tile([COUT, K], F32, tag="btmp")
                bacc = singles.tile([COUT, 1], F32, tag="bacc")
                nc.vector.scalar_tensor_tensor(
                    out=btmp, in0=biasT, scalar=1.0, in1=alpha,
                    op0=ALU.mult, op1=ALU.mult, accum_out=bacc)
                nc.vector.tensor_scalar_mul(beff[:, b:b + 1], bacc, rSs[b])

            def conv_chunk(b, hc, mid_work):
                """One 4-row output chunk: 9 accumulating matmuls -> PSUM,
                evacuation (ACT on even chunks, DVE on odd), paired output
                DMA on the gpsimd ring.  mid_work() emits interleaved
                program-order work (x1 prep / router-1) after the matmuls."""
                wfA, wfB = weffs[b]
                xp = xpads[b]
                ps = pconv.tile([COUT, FREE], F32, tag="ps")
                for j in range(JT):
                    dy, dx = divmod(j, KS)
                    wf = wfA[:, j, :] if j < JA else wfB[:, j - JA, :]
                    nc.tensor.matmul(
                        ps, wf,
                        xp[:, RPC * hc + dy:RPC * hc + dy + RPC, dx:dx + W],
                        start=(j == 0), stop=(j == JT - 1))
                if mid_work is not None:
                    mid_work()
                if hc % 2 == 0:
                    self_ot = outp.tile([COUT, 2 * FREE], F32, tag="ot")
                    conv_chunk.ot = self_ot
                ot = conv_chunk.ot
                half = ot[:, (hc % 2) * FREE:(hc % 2 + 1) * FREE]
                if hc % 2 == 0:
                    nc.scalar.activation(out=half, in_=ps, func=AF.Identity,
                                         bias=beff[:, b:b + 1], scale=rSs[b])
                else:
                    nc.vector.scalar_tensor_tensor(
                        out=half, in0=ps, scalar=rSs[b],
                        in1=beff[:, b:b + 1].broadcast_to([COUT, FREE]),
                        op0=ALU.mult, op1=ALU.add)
                    nc.gpsimd.dma_start(
                        out=y_d[b, :, RPC * (hc - 1):RPC * (hc + 1), :],
                        in_=ot.rearrange("p (r w) -> p r w", w=W))

            # ---- program ------------------------------------------------
            make_xpad(0)
            make_xpad(1)
            x0_stages = issue_slab_dmas(0)
            x1_stages = issue_slab_dmas(1)

            # replicated consts ride the gpsimd ring (ACT stays free for
            # casts, sync for the x slabs)
            eT = singles.tile([CIN, K, JT, COUT], F32)
            et_flat = eT.rearrange("p k j co -> p k (j co)")
            nc.gpsimd.dma_start(out=eT, in_=et_d[:, :, :, :])

            ones4 = singles.tile([K, 128], F32)
            ones4_d = nc.inline_tensor(np.ones((K, 128), np.float32),
                                       name="ones4_const")
            nc.gpsimd.dma_start(out=ones4, in_=ones4_d[:, :])

            sel_np = np.zeros((K, K, 128), np.float32)
            for k in range(K):
                sel_np[k, k, :] = 1.0
            sel = singles.tile([K, K, 128], F32)
            sel_d = nc.inline_tensor(sel_np, name="sel_const")
            nc.gpsimd.dma_start(out=sel, in_=sel_d[:, :, :])

            b1t = singles.tile([HID, 1], F32)
            nc.gpsimd.dma_start(out=b1t, in_=b1_d[:].unsqueeze(-1))
            b2t = singles.tile([K, 1], F32)
            nc.gpsimd.dma_start(out=b2t, in_=b2_d[:].unsqueeze(-1))
            w1t = singles.tile([CIN, HID], F32)
            nc.gpsimd.dma_start(out=w1t, in_=w1t_d[:, :])
            w2t = singles.tile([HID, K], F32)
            nc.gpsimd.dma_start(out=w2t, in_=w2t_d[:, :])
            biasT = singles.tile([COUT, K], F32)
            nc.gpsimd.dma_start(out=biasT, in_=biast_d[:, :])

            for s in range(NSLAB):
                prep_slab(0, s, x0_stages[s])

            route(0)
            weff_block(0)

            # conv-0 with x1 prep and router-1 injected mid-stream
            def make_mid(hc):
                def mid():
                    if hc % 2 == 0 and hc // 2 < NSLAB:
                        s = hc // 2
                        prep_slab(1, s, x1_stages[s])
                    if hc == 16:
                        route(1)
                    if hc == 17:
                        weff_block(1)
                return mid

            for hc in range(NCHUNK):
                conv_chunk(0, hc, make_mid(hc))
            for hc in range(NCHUNK):
                conv_chunk(1, hc, None)

    _legalize_waits(nc)
    return nc


_NC_CACHE = None


def get_nc() -> bass.Bass:
    global _NC_CACHE
    if _NC_CACHE is None:
        _NC_CACHE = build_nc()
    return _NC_CACHE


def make_in_maps(inputs: dict[str, np.ndarray]) -> list[dict[str, np.ndarray]]:
    x = np.ascontiguousarray(np.asarray(inputs["x"], dtype=np.float32))
    experts = np.asarray(inputs["experts"], np.float32)
    # host-side layout prep (no math): experts -> lhsT layout [ci, k, j, co];
    # w1 additionally folds the 1/(H*W) mean divisor into its transpose
    et = np.ascontiguousarray(
        experts.reshape(K, COUT, CIN, JT).transpose(2, 0, 3, 1))
    shared = {
        "experts_t": et,
        "bias_t": np.ascontiguousarray(
            np.asarray(inputs["bias"], np.float32).T),
        "w1t": np.ascontiguousarray(
            np.asarray(inputs["w1"], np.float32).T / float(H * W)),
        "b1": np.ascontiguousarray(np.asarray(inputs["b1"], np.float32)),
        "w2t": np.ascontiguousarray(np.asarray(inputs["w2"], np.float32).T),
        "b2": np.ascontiguousarray(np.asarray(inputs["b2"], np.float32)),
    }
    return [
        {"x": x[c * BPC:(c + 1) * BPC], **shared}
        for c in range(N_CORES)
    ]


def kernel(**inputs: np.ndarray) -> np.ndarray:
    nc = get_nc()
    res = bass_utils.run_bass_kernel_spmd(
        nc, make_in_maps(inputs), core_ids=list(range(N_CORES)),
    )
    return np.concatenate(
        [res.results[c]["y"] for c in range(N_CORES)], axis=0)


# revision 5
# speedup vs baseline: 1.0160x; 1.0160x over previous
"""CondConv2d (MoE-routed 3x3 conv) Trainium2 Bass kernel.

Problem (hardcoded shapes):
  x:       (16, 128, 128, 128) f32   B, C_in, H, W
  experts: (4, 128, 128, 3, 3) f32   K, C_out, C_in, kh, kw
  bias:    (4, 128) f32              K, C_out
  w1:      (32, 128) f32             HID, C_in
  b1:      (32,) f32
  w2:      (4, 32) f32               K, HID
  b2:      (4,) f32
  out:     (16, 128, 128, 128) f32   B, C_out, H, W  (stride 1, pad 1)

Sharding: data-parallel over batch, 2 samples per core x 8 cores; the tiny
expert/router params are replicated (pre-transposed on the host into the
matmul-friendly layouts -- pure layout prep, all math stays on device).

Per-core schedule (PE-bound at ~216ns per 512-col fp16 matmul):
  0-24us   x0 streams in (DMA-bound).  A dense stream of zero matmuls keeps
           the PE HAM activity window busy so the conv starts at 2.4GHz
           instead of 1.2GHz.  DVE chases slabs with channel sums, ACT casts
           fp32->fp16 into the padded image.  x1 slab DMAs are issued up
           front too: the sync ring drains them right after x0's.
  ~26us    router-0 (tiny PE matmuls) -> weff-0 on DVE, split into two tap
           halves so the first conv matmuls start ~2us earlier.
  26-88us  conv-0: 32 chunks x 9 accumulating matmuls (PSUM), evacuation
           alternates ACT/DVE, output DMA per 2 chunks on the idle GpSimd
           ring (a dynamic-DMA trigger costs ~0.8us of issuing-engine time,
           which previously saturated ACT).  x1's reduce/cast ops are
           interleaved into the conv program order so they never block
           conv evacuations on the FIFO engines; router-1 runs mid-stream
           after chunk 16 (x1 has landed by then), so the PE never waits.
  88-152us conv-1, same structure.
"""

import numpy as np

import concourse.bass as bass
import concourse.mybir as mybir
import concourse.tile as tile
from concourse import bass_utils


def _legalize_waits(nc, keep=1):
    """This container's walrus rejects >1 sync wait per instruction
    (setupSyncWait: "Too many sync wait commands").  Hoist extra waits into
    standalone EventSemaphore wait-nops on the same engine, which is what
    raw-bass wait_ge() emits; ">=" waits commute so order doesn't matter."""
    counter = [0]

    def fix_block(block):
        out, changed = [], False
        for inst in block.instructions:
            si = inst.sync_info
            waits = list(si.on_wait) if si is not None else []
            if len(waits) > keep:
                for w in waits[:-keep]:
                    nm = f"{inst.name}-w{counter[0]}"
                    counter[0] += 1
                    nop = mybir.InstEventSemaphore(name=nm, ins=[], outs=[])
                    nop.engine = inst.engine
                    nop.sync_info = mybir.SyncInfo(on_wait=[w], on_update=[])
                    nc.inst_map[nm] = nop
                    out.append(nop)
                inst.sync_info = mybir.SyncInfo(
                    on_wait=waits[-keep:], on_update=list(si.on_update)
                )
                changed = True
            out.append(inst)
        if changed:
            block.instructions = out
        for sub in getattr(block, "blocks", []) or []:
            fix_block(sub)

    for fn in nc.m.functions:
        for b in fn.blocks:
            fix_block(b)


F32 = mybir.dt.float32
F16 = mybir.dt.float16
AF = mybir.ActivationFunctionType
ALU = mybir.AluOpType

B, CIN, COUT, K, KS, H, W, HID = 16, 128, 128, 4, 3, 128, 128, 32
N_CORES = 8
BPC = B // N_CORES          # samples per core
HP, WP = H + 2, W + 2       # zero-padded image
RPC = 4                     # output rows per chunk
NCHUNK = H // RPC           # 32 chunks per sample
FREE = RPC * W              # 512 = matmul moving free size (one PSUM bank)
NSLAB = 8                   # x-load slabs per sample (16 rows each)
SLAB_ROWS = H // NSLAB
JT = KS * KS                # 9 taps
JA = 5                      # taps in the first weff half
WARMUP_MMS = 95            # junk matmuls covering the x0 load window


def build_nc() -> bass.Bass:
    nc = bass.Bass(trn_type="TRN2", target_bir_lowering=False, debug=False)

    x_d = nc.dram_tensor("x", [BPC, CIN, H, W], F32, kind="ExternalInput")
    et_d = nc.dram_tensor("experts_t", [CIN, K, JT, COUT], F32,
                          kind="ExternalInput")
    biast_d = nc.dram_tensor("bias_t", [COUT, K], F32, kind="ExternalInput")
    w1t_d = nc.dram_tensor("w1t", [CIN, HID], F32, kind="ExternalInput")
    b1_d = nc.dram_tensor("b1", [HID], F32, kind="ExternalInput")
    w2t_d = nc.dram_tensor("w2t", [HID, K], F32, kind="ExternalInput")
    b2_d = nc.dram_tensor("b2", [K], F32, kind="ExternalInput")
    y_d = nc.dram_tensor("y", [BPC, COUT, H, W], F16, kind="ExternalOutput")

    with tile.TileContext(nc) as tc:
        with (
            tc.tile_pool(name="singles", bufs=1) as singles,
            tc.tile_pool(name="stage", bufs=6) as stage_pool,
            tc.tile_pool(name="stage1", bufs=3) as stage1_pool,
            tc.tile_pool(name="outp", bufs=4) as outp,
            tc.tile_pool(name="pconv", bufs=6, space="PSUM") as pconv,
            tc.tile_pool(name="prt", bufs=2, space="PSUM") as prt,
        ):
            xpads = [None] * BPC
            weffs = [None] * BPC      # (wfA taps 0..4, wfB taps 5..8)
            alphas = [None] * BPC
            rSs = [None] * BPC
            partials_t = [None] * BPC
            beff = singles.tile([COUT, BPC], F32)

            # ---- PE warmup: dense junk matmuls under the x0 load ---------
            warm_w = singles.tile([CIN, COUT], F16)
            warm_x = singles.tile([CIN, FREE], F16)
            nc.vector.memset(warm_w, 0.0)
            nc.vector.memset(warm_x, 0.0)
            for _ in range(WARMUP_MMS):
                wps = pconv.tile([COUT, FREE], F32, tag="ps")
                nc.tensor.matmul(wps, warm_w, warm_x)

            def make_xpad(b):
                xp = singles.tile([CIN, HP, WP], F16, tag=f"xpad{b}",
                                  name=f"xpad{b}")
                xpads[b] = xp
                nc.vector.memset(xp[:, 0, :], 0.0)
                nc.vector.memset(xp[:, HP - 1, :], 0.0)
                nc.vector.memset(xp[:, :, 0], 0.0)
                nc.vector.memset(xp[:, :, WP - 1], 0.0)
                partials_t[b] = singles.tile(
                    [CIN, NSLAB], F32, tag=f"partials{b}", name=f"partials{b}")

            def issue_slab_dmas(b):
                """Queue all of sample b's slab DMAs on the sync ring; the
                ring drains them in order (stage-pool reuse throttles via
                HWDGE semaphore waits)."""
                stages, insts = [], []
                pool = stage_pool if b == 0 else stage1_pool
                for s in range(NSLAB):
                    r0 = s * SLAB_ROWS
                    stage = pool.tile([CIN, SLAB_ROWS, W], F32,
                                      tag="stage")
                    di = nc.sync.dma_start(
                        out=stage, in_=x_d[b, :, r0:r0 + SLAB_ROWS, :])
                    stages.append(stage)
                    insts.append(di)
                return stages, insts

            def prep_slab(b, s, stage):
                """One fused op per slab: fp32->fp16 cast into the padded
                image with the channel-sum riding along as accum_out (fp32).
                Even slabs on ACT, odd on DVE so parallel-landing slabs
                drain twice as fast."""
                r0 = s * SLAB_ROWS
                dst = xpads[b][:, 1 + r0:1 + r0 + SLAB_ROWS, 1:1 + W]
                acc = partials_t[b][:, s:s + 1]
                if s % 2 == 0:
                    nc.scalar.activation(out=dst, in_=stage, func=AF.Copy,
                                         accum_out=acc)
                else:
                    with nc.allow_low_precision(reason="fp16 conv input"):
                        nc.vector.tensor_scalar(
                            out=dst, in0=stage, scalar1=1.0, scalar2=0.0,
                            op0=ALU.mult, op1=ALU.add, accum_out=acc)

            def route(b):
                """Router MLP + softmax through broadcast alpha."""
                partials = partials_t[b]
                gT = singles.tile([CIN, 1], F32, tag=f"gT{b}", name=f"gT{b}")
                nc.vector.tensor_reduce(
                    out=gT, in_=partials, axis=mybir.AxisListType.X,
                    op=ALU.add)

                h_ps = prt.tile([HID, 1], F32, tag="rt")
                nc.tensor.matmul(h_ps, w1t, gT)
                h_sb = singles.tile([HID, 1], F32, tag=f"h_sb{b}",
                                    name=f"h_sb{b}")
                nc.scalar.activation(out=h_sb, in_=h_ps, func=AF.Relu,
                                     bias=b1t)

                lg_ps = prt.tile([K, 1], F32, tag="rt")
                nc.tensor.matmul(lg_ps, w2t, h_sb)
                # expl = exp(logits + b2); logits are tiny, no max-sub needed
                expl = singles.tile([K, 1], F32, tag=f"expl{b}",
                                    name=f"expl{b}")
                nc.scalar.activation(out=expl, in_=lg_ps, func=AF.Exp,
                                     bias=b2t)

                # broadcast expl[k] (unnormalized) to all partitions via
                # selector matmuls; the softmax 1/sum normalization is
                # applied later as the PSUM-evacuation scale
                ab_ps = prt.tile([128, K], F32, tag="rt")
                for k in range(K):
                    nc.tensor.matmul(ab_ps[:, k:k + 1], sel[:, k, :], expl)
                alpha = singles.tile([128, K], F32, tag=f"alpha{b}",
                                     name=f"alpha{b}")
                nc.vector.tensor_copy(out=alpha, in_=ab_ps)
                alphas[b] = alpha

                den_ps = prt.tile([128, 1], F32, tag="rt")
                nc.tensor.matmul(den_ps, ones4, expl)
                rS = singles.tile([128, 1], F32, tag=f"rS{b}", name=f"rS{b}")
                nc.vector.reciprocal(out=rS, in_=den_ps)
                rSs[b] = rS

            def weff_block(b):
                """weff_b = sum_k alpha[k] * expertT_k (fp32 accumulate,
                fp16 result), built in two tap halves so conv tap-0 matmuls
                can start before the full chain finishes."""
                alpha = alphas[b]
                halves = []
                for (name, j0, j1) in (("A", 0, JA), ("B", JA, JT)):
                    nj = j1 - j0
                    wf = singles.tile([CIN, nj, COUT], F16,
                                      tag=f"weff{name}{b}",
                                      name=f"weff{name}{b}")
                    wf_flat = wf.rearrange("p j co -> p (j co)")
                    src = et_flat[:, :, j0 * COUT:j1 * COUT]
                    with nc.allow_low_precision(reason="fp16 conv weights"):
                        nc.vector.tensor_scalar_mul(
                            wf_flat, src[:, 0, :], alpha[:, 0:1])
                        for k in range(1, K):
                            nc.vector.scalar_tensor_tensor(
                                out=wf_flat, in0=src[:, k, :],
                                scalar=alpha[:, k:k + 1], in1=wf_flat,
                                op0=ALU.mult, op1=ALU.add)
                    halves.append(wf)
                weffs[b] = halves

                btmp = singles.tile([COUT, K], F32, tag="btmp")
                bacc = singles.tile([COUT, 1], F32, tag="bacc")
                nc.vector.scalar_tensor_tensor(
                    out=btmp, in0=biasT, scalar=1.0, in1=alpha,
                    op0=ALU.mult, op1=ALU.mult, accum_out=bacc)
                nc.vector.tensor_scalar_mul(beff[:, b:b + 1], bacc, rSs[b])

            def conv_chunk(b, hc, mid_work):
                """One 4-row output chunk: 9 accumulating matmuls -> PSUM,
                evacuation (ACT on even chunks, DVE on odd), paired output
                DMA on the gpsimd ring.  mid_work() emits interleaved
                program-order work (x1 prep / router-1) after the matmuls."""
                wfA, wfB = weffs[b]
                xp = xpads[b]
                ps = pconv.tile([COUT, FREE], F32, tag="ps")
                for j in range(JT):
                    dy, dx = divmod(j, KS)
                    wf = wfA[:, j, :] if j < JA else wfB[:, j - JA, :]
                    nc.tensor.matmul(
                        ps, wf,
                        xp[:, RPC * hc + dy:RPC * hc + dy + RPC, dx:dx + W],
                        start=(j == 0), stop=(j == JT - 1))
                if mid_work is not None:
                    mid_work()
                if hc % 2 == 0:
                    self_ot = outp.tile([COUT, 2 * FREE], F16, tag="ot")
                    conv_chunk.ot = self_ot
                ot = conv_chunk.ot
                half = ot[:, (hc % 2) * FREE:(hc % 2 + 1) * FREE]
                if hc % 2 == 0:
                    nc.scalar.activation(out=half, in_=ps, func=AF.Identity,
                                         bias=beff[:, b:b + 1], scale=rSs[b])
                else:
                    with nc.allow_low_precision(reason="fp16 output"):
                        nc.vector.scalar_tensor_tensor(
                            out=half, in0=ps, scalar=rSs[b],
                            in1=beff[:, b:b + 1].broadcast_to([COUT, FREE]),
                            op0=ALU.mult, op1=ALU.add)
                    nc.gpsimd.dma_start(
                        out=y_d[b, :, RPC * (hc - 1):RPC * (hc + 1), :],
                        in_=ot.rearrange("p (r w) -> p r w", w=W))

            # ---- program ------------------------------------------------
            make_xpad(0)
            make_xpad(1)
            x0_stages, x0_dmas = issue_slab_dmas(0)
            x1_stages, x1_dmas = issue_slab_dmas(1)

            # replicated consts ride the gpsimd ring (ACT stays free for
            # casts, sync for the x slabs)
            eT = singles.tile([CIN, K, JT, COUT], F32)
            et_flat = eT.rearrange("p k j co -> p k (j co)")
            nc.gpsimd.dma_start(out=eT, in_=et_d[:, :, :, :])

            ones4 = singles.tile([K, 128], F32)
            ones4_d = nc.inline_tensor(np.ones((K, 128), np.float32),
                                       name="ones4_const")
            nc.gpsimd.dma_start(out=ones4, in_=ones4_d[:, :])

            sel_np = np.zeros((K, K, 128), np.float32)
            for k in range(K):
                sel_np[k, k, :] = 1.0
            sel = singles.tile([K, K, 128], F32)
            sel_d = nc.inline_tensor(sel_np, name="sel_const")
            nc.gpsimd.dma_start(out=sel, in_=sel_d[:, :, :])

            b1t = singles.tile([HID, 1], F32)
            nc.gpsimd.dma_start(out=b1t, in_=b1_d[:].unsqueeze(-1))
            b2t = singles.tile([K, 1], F32)
            nc.gpsimd.dma_start(out=b2t, in_=b2_d[:].unsqueeze(-1))
            w1t = singles.tile([CIN, HID], F32)
            nc.gpsimd.dma_start(out=w1t, in_=w1t_d[:, :])
            w2t = singles.tile([HID, K], F32)
            nc.gpsimd.dma_start(out=w2t, in_=w2t_d[:, :])
            biasT = singles.tile([COUT, K], F32)
            nc.gpsimd.dma_start(out=biasT, in_=biast_d[:, :])

            for s in range(NSLAB):
                prep_slab(0, s, x0_stages[s])

            route(0)
            weff_block(0)

            # conv-0 with x1 prep and router-1 injected mid-stream
            def make_mid(hc):
                def mid():
                    if hc % 2 == 0 and 4 <= hc < 4 + 2 * NSLAB:
                        s = (hc - 4) // 2
                        prep_slab(1, s, x1_stages[s])
                    if hc == 22:
                        route(1)
                    if hc == 23:
                        weff_block(1)
                return mid

            for hc in range(NCHUNK):
                conv_chunk(0, hc, make_mid(hc))
            for hc in range(NCHUNK):
                conv_chunk(1, hc, None)

    _legalize_waits(nc)
    return nc


_NC_CACHE = None


def get_nc() -> bass.Bass:
    global _NC_CACHE
    if _NC_CACHE is None:
        _NC_CACHE = build_nc()
    return _NC_CACHE


def make_in_maps(inputs: dict[str, np.ndarray]) -> list[dict[str, np.ndarray]]:
    x = np.ascontiguousarray(np.asarray(inputs["x"], dtype=np.float32))
    experts = np.asarray(inputs["experts"], np.float32)
    # host-side layout prep (no math): experts -> lhsT layout [ci, k, j, co];
    # w1 additionally folds the 1/(H*W) mean divisor into its transpose
    et = np.ascontiguousarray(
        experts.reshape(K, COUT, CIN, JT).transpose(2, 0, 3, 1))
    shared = {
        "experts_t": et,
        "bias_t": np.ascontiguousarray(
            np.asarray(inputs["bias"], np.float32).T),
        "w1t": np.ascontiguousarray(
            np.asarray(inputs["w1"], np.float32).T / float(H * W)),
        "b1": np.ascontiguousarray(np.asarray(inputs["b1"], np.float32)),
        "w2t": np.ascontiguousarray(np.asarray(inputs["w2"], np.float32).T),
        "b2": np.ascontiguousarray(np.asarray(inputs["b2"], np.float32)),
    }
    return [
        {"x": x[c * BPC:(c + 1) * BPC], **shared}
        for c in range(N_CORES)
    ]


def kernel(**inputs: np.ndarray) -> np.ndarray:
    nc = get_nc()
    res = bass_utils.run_bass_kernel_spmd(
        nc, make_in_maps(inputs), core_ids=list(range(N_CORES)),
    )
    return np.concatenate(
        [res.results[c]["y"].astype(np.float32) for c in range(N_CORES)],
        axis=0)
